# revision 1
# baseline (speedup 1.0000x reference)
"""Trainium2 Bass kernel for BoW: embedding gather + ragged segment-sum + Linear + ReLU.

Strategy (8 NeuronCores, data-parallel over sentences):
  - Core c owns segments [c*2048, (c+1)*2048). Tokens are split at sentence
    boundaries (segment_ids is sorted), so no cross-core reduction is needed.
  - Per core, tokens are grouped by (segment-window, vocab-shard):
      * segment-window: PSUM accumulator window of WIN segments (matmul target)
      * vocab-shard: dma_gather uses int16 indices, so the 100k-row table is
        addressed in shards of 32768 rows
    Groups are padded to a uniform size across all 8 cores so a single SPMD
    program serves every core (pad tokens gather row 0 and carry segment -1,
    which never matches the one-hot compare, so they contribute zero).
  - Embedding rows are fetched with dma_gather (bulk indirect DMA). For each
    128-token chunk, a one-hot matrix onehot[t, s] = (seg[t] == s) is built on
    VectorE via is_equal against an iota row, and TensorE accumulates
      bowT[d, s] += emb[t, d]^T @ onehot[t, s]
    into PSUM. Finally each 128-segment tile is multiplied by W, bias-added,
    ReLU'd, and DMA'd out.
"""

import numpy as np

N_CORES = 8
NSEG_TOTAL = 16384
SHARD_ROWS = 32768  # int16 gather index range
CHUNK = 128


def _prep_host(tokens, segment_ids, vocab, nseg, win, shard_rows):
    """Group tokens per core by (segment window, vocab shard); pad to uniform sizes.

    Returns per-core streams (int16 gather indices, f32 window-relative segment
    ids) plus the shared group size table.
    """
    seg_per_core = nseg // N_CORES
    nst = seg_per_core // win
    n_shards = (vocab + shard_rows - 1) // shard_rows
    ngroups = nst * n_shards

    bounds = np.searchsorted(segment_ids, np.arange(N_CORES + 1) * seg_per_core)
    per_core = []
    counts = np.zeros((N_CORES, ngroups), dtype=np.int64)
    for c in range(N_CORES):
        lo, hi = bounds[c], bounds[c + 1]
        tok = tokens[lo:hi].astype(np.int64)
        seg = segment_ids[lo:hi].astype(np.int64) - c * seg_per_core
        st = seg // win
        sh = tok // shard_rows
        key = st * n_shards + sh
        counts[c] = np.bincount(key, minlength=ngroups)
        per_core.append((tok, seg, key))

    G = counts.max(axis=0)
    G = ((G + CHUNK - 1) // CHUNK) * CHUNK
    # Ensure every segment window gets at least one chunk so its PSUM region
    # is written (all-pad chunk writes zeros, which is the correct sum).
    for st in range(nst):
        if G[st * n_shards : (st + 1) * n_shards].sum() == 0:
            G[st * n_shards] = CHUNK
    off = np.concatenate([[0], np.cumsum(G)])
    tot = int(off[-1])

    idx_hw, segf_hw = [], []
    for c in range(N_CORES):
        tok, seg, key = per_core[c]
        n = tok.shape[0]
        idx_stream = np.zeros(tot, dtype=np.int16)
        segf_stream = np.full(tot, -1.0, dtype=np.float32)
        # sort by (group, token id): token-sorted gathers walk the table
        # near-monotonically (HBM row-buffer locality)
        order = np.lexsort((tok, key))
        key_sorted = key[order]
        group_start = np.searchsorted(key_sorted, np.arange(ngroups))
        rank = np.arange(n) - group_start[key_sorted]
        dest = off[key_sorted] + rank
        idx_stream[dest] = (tok[order] % shard_rows).astype(np.int16)
        segf_stream[dest] = (seg[order] - (key_sorted // n_shards) * win).astype(
            np.float32
        )
        # wrap by 16 partitions, replicate for the 8 gpsimd cores
        idx16 = np.tile(
            np.ascontiguousarray(idx_stream.reshape(tot // 16, 16).T), (8, 1)
        )
        segf = np.ascontiguousarray(segf_stream.reshape(tot // CHUNK, CHUNK).T)
        idx_hw.append(np.ascontiguousarray(idx16))
        segf_hw.append(segf)

    return idx_hw, segf_hw, G, off, tot, nst, n_shards


LAST_RESULT = None  # BassKernelResults of the most recent run (for profiling)
LAST_NC = None
LAST_IN_MAPS = None


def _build_program(G, off, tot, nst, n_shards, win, nseg, mm_dtype_name,
                   shard_rows, vocab, dim, reps=1, parts="all", n_queues=1,
                   gmax=None):
    """Build the (core-uniform) SPMD Bass program. Returns the compiled nc."""
    import concourse.bacc as bacc
    import concourse.mybir as mybir
    from concourse.tile import TileContext

    f32 = mybir.dt.float32
    i16 = mybir.dt.int16
    # "mixed": gather f32 rows (512B descriptors run ~2x faster than 256B),
    # cast to bf16 on-chip, run bf16 matmuls. Otherwise table dtype == compute
    # dtype. float32r tiles must be declared as such (verifier wants rounded
    # producers).
    if mm_dtype_name == "mixed":
        mm_dt = mybir.dt.bfloat16
        tbl_dt = f32
    else:
        mm_dt = getattr(mybir.dt, mm_dtype_name)
        tbl_dt = mm_dt
    seg_per_core = nseg // N_CORES
    nchunks = tot // CHUNK

    nc = bacc.Bacc("TRN2", num_devices=N_CORES, num_swdge_queues=n_queues)
    tbl_d = nc.declare_dram_parameter("tbl", [vocab, dim], tbl_dt, isOutput=False)
    idx_d = nc.declare_dram_parameter("idx", [128, tot // 16], i16, isOutput=False)
    segf_d = nc.declare_dram_parameter("segf", [128, nchunks], f32, isOutput=False)
    iota_d = nc.declare_dram_parameter("iota", [128, win], f32, isOutput=False)
    w_d = nc.declare_dram_parameter("w", [dim, dim], f32, isOutput=False)
    brep_d = nc.declare_dram_parameter("brep", [128, dim], f32, isOutput=False)
    out_d = nc.declare_dram_parameter("out", [seg_per_core, dim], f32, isOutput=True)

    with TileContext(nc) as tc:
        with (
            tc.tile_pool(name="const", bufs=1) as cpool,
            tc.tile_pool(name="emb", bufs=5) as epool,
            tc.tile_pool(name="oh", bufs=3) as ohpool,
            tc.tile_pool(name="bow", bufs=1, space="PSUM") as bowpool,
            tc.tile_pool(name="o2", bufs=2, space="PSUM") as o2pool,
            tc.tile_pool(name="tail", bufs=3) as tailpool,
        ):
            idx_sb = cpool.tile([128, tot // 16], i16)
            nc.sync.dma_start(out=idx_sb[:], in_=idx_d[:])
            segf_sb = cpool.tile([128, nchunks], mm_dt)
            if mm_dt == f32:
                nc.sync.dma_start(out=segf_sb[:], in_=segf_d[:])
            else:
                segf_f32 = cpool.tile([128, nchunks], f32)
                nc.sync.dma_start(out=segf_f32[:], in_=segf_d[:])
                nc.vector.tensor_copy(out=segf_sb[:], in_=segf_f32[:])
            iota_sb = cpool.tile([128, win], mm_dt)
            if mm_dt == f32:
                nc.sync.dma_start(out=iota_sb[:], in_=iota_d[:])
            else:
                iota_f32 = cpool.tile([128, win], f32)
                nc.sync.dma_start(out=iota_f32[:], in_=iota_d[:])
                nc.vector.tensor_copy(out=iota_sb[:], in_=iota_f32[:])
            w_sb = cpool.tile([dim, dim], f32)
            nc.sync.dma_start(out=w_sb[:], in_=w_d[:])
            brep_sb = cpool.tile([128, dim], f32)
            nc.sync.dma_start(out=brep_sb[:], in_=brep_d[:])

            bow = None
            if parts != "gather":
                bow = bowpool.tile([128, seg_per_core], f32, tag="bow")
            dummy_emb = None
            if parts == "compute":
                dummy_emb = cpool.tile([128, dim], mm_dt, tag="dummy_emb")
                nc.vector.memset(dummy_emb[:], 0.0)

            max_blk = int(G.max()) // CHUNK

            _gq = [0]  # gather counter for queue round-robin

            def emit_body():
                gc = 0
                for st in range(nst):
                    first_mm_of_win = True
                    # last group index in this window with G>0
                    live = [s for s in range(n_shards) if G[st * n_shards + s] > 0]
                    for sh in range(n_shards):
                        j = st * n_shards + sh
                        gj = int(G[j])
                        if gj == 0:
                            continue
                        nblk = gj // CHUNK
                        emb = None
                        if parts != "compute":
                            emb = epool.tile([128, max_blk, dim], tbl_dt, tag="emb")
                        rows = min(shard_rows, vocab - sh * shard_rows)
                        if parts in ("all", "gather"):
                            step = nblk if gmax is None else max(1, gmax // CHUNK)
                            for sub in range(0, nblk, step):
                                k = min(step, nblk - sub)
                                sgj = k * CHUNK
                                o0 = int(off[j]) + sub * CHUNK
                                nc.gpsimd.dma_gather(
                                    emb[:, sub : sub + k, :],
                                    tbl_d[
                                        sh * shard_rows : sh * shard_rows + rows, :
                                    ],
                                    idx_sb[:, o0 // 16 : (o0 + sgj) // 16],
                                    num_idxs=sgj,
                                    num_idxs_reg=sgj,
                                    elem_size=dim,
                                    single_packet=(sgj <= 1008),
                                    queue_num=_gq[0] % n_queues,
                                )
                                _gq[0] += 1
                        if parts == "gather":
                            continue
                        mm_emb = emb
                        if tbl_dt != mm_dt and parts != "compute":
                            # cast gathered rows f32 -> bf16; alternate DVE/ACT
                            # so the cast stays off whichever engine is busier
                            mm_emb = epool.tile(
                                [128, max_blk, dim], mm_dt, tag="emb16"
                            )
                            nc.scalar.copy(
                                out=mm_emb[:, :nblk, :], in_=emb[:, :nblk, :]
                            )
                        # one-hot in sub-group batches: oh[t, c, s] =
                        # (segf[t, gc+c] == iota[s]). Smaller tiles than
                        # one-per-group keep SBUF free for gather buffers.
                        oh_blk = min(max_blk, 10)
                        for ob in range(0, nblk, oh_blk):
                            kb = min(oh_blk, nblk - ob)
                            ohg = ohpool.tile([128, oh_blk, win], mm_dt, tag="oh")
                            seg_b = segf_sb[
                                :, gc + ob : gc + ob + kb
                            ].broadcast_to([128, kb, win])
                            iota_b = iota_sb[:].rearrange(
                                "p (a w) -> p a w", a=1
                            ).broadcast_to([128, kb, win])
                            nc.vector.tensor_tensor(
                                out=ohg[:, :kb, :],
                                in0=iota_b,
                                in1=seg_b,
                                op=mybir.AluOpType.is_equal,
                            )
                            for cblk in range(ob, ob + kb):
                                is_last = sh == live[-1] and cblk == nblk - 1
                                lhsT = (
                                    dummy_emb[:] if parts == "compute"
                                    else mm_emb[:, cblk, :]
                                )
                                nc.tensor.matmul(
                                    out=bow[:, st * win : (st + 1) * win],
                                    lhsT=lhsT,
                                    rhs=ohg[:, cblk - ob, :],
                                    start=first_mm_of_win,
                                    stop=is_last,
                                )
                                first_mm_of_win = False
                        gc += nblk

                for ot in range(seg_per_core // 128 if parts != "gather" else 0):
                    bsb = tailpool.tile([128, 128], f32, tag="bsb")
                    nc.vector.tensor_copy(
                        out=bsb[:], in_=bow[:, ot * 128 : (ot + 1) * 128]
                    )
                    o2 = o2pool.tile([128, dim], f32, tag="o2")
                    nc.tensor.matmul(
                        out=o2[:], lhsT=bsb[:], rhs=w_sb[:], start=True, stop=True
                    )
                    osb = tailpool.tile([128, dim], f32, tag="osb")
                    nc.vector.tensor_tensor(
                        out=osb[:], in0=o2[:], in1=brep_sb[:], op=mybir.AluOpType.add
                    )
                    nc.scalar.activation(
                        out=osb[:], in_=osb[:], func=mybir.ActivationFunctionType.Relu
                    )
                    nc.sync.dma_start(
                        out=out_d[ot * 128 : (ot + 1) * 128, :], in_=osb[:]
                    )

            for _ in range(reps):
                emit_body()

    nc.compile()
    return nc


def _make_in_maps(idx_hw, segf_hw, table, W, b, win, mm_dtype_name):
    import concourse.mybir as mybir

    if mm_dtype_name == "mixed":
        tbl_dt = mybir.dt.float32
    else:
        tbl_dt = getattr(mybir.dt, mm_dtype_name)
    iota_hw = np.tile(np.arange(win, dtype=np.float32), (128, 1))
    brep_hw = np.tile(b.astype(np.float32), (128, 1))
    tbl_np = np.ascontiguousarray(table.astype(mybir.dt.np(tbl_dt)))
    return [
        {
            "tbl": tbl_np,
            "idx": idx_hw[c],
            "segf": segf_hw[c],
            "iota": iota_hw,
            "w": np.ascontiguousarray(W.astype(np.float32)),
            "brep": brep_hw,
        }
        for c in range(N_CORES)
    ]


def kernel(tokens, segment_ids, embedding_table, W, b, *, nseg=NSEG_TOTAL, win=256,
           mm_dtype="float32r", shard_rows=SHARD_ROWS, trace=False, n_queues=4,
           gmax=896):
    from concourse.bass_utils import run_bass_kernel_spmd

    tokens = np.asarray(tokens, dtype=np.int32)
    segment_ids = np.asarray(segment_ids, dtype=np.int32)
    embedding_table = np.asarray(embedding_table, dtype=np.float32)
    W = np.asarray(W, dtype=np.float32)
    b = np.asarray(b, dtype=np.float32)
    vocab, dim = embedding_table.shape

    idx_hw, segf_hw, G, off, tot, nst, n_shards = _prep_host(
        tokens, segment_ids, vocab, nseg, win, shard_rows
    )
    nc = _build_program(G, off, tot, nst, n_shards, win, nseg, mm_dtype,
                        shard_rows, vocab, dim, n_queues=n_queues, gmax=gmax)
    in_maps = _make_in_maps(idx_hw, segf_hw, embedding_table, W, b, win, mm_dtype)
    res = run_bass_kernel_spmd(
        nc, in_maps, core_ids=list(range(N_CORES)), trace=trace
    )
    global LAST_RESULT, LAST_NC, LAST_IN_MAPS
    LAST_RESULT = res
    LAST_NC = nc
    LAST_IN_MAPS = in_maps
    return np.concatenate([res.results[c]["out"] for c in range(N_CORES)], axis=0)

